# revision 5
# baseline (speedup 1.0000x reference)
"""Trainium2 Bass kernel for nn_AttentionLayer (B=8, S=2048, D=512).

Sharding: pure data parallel — batch b runs on core b (8 batches, 8 cores,
no collectives). Per core: out = softmax(Q @ K^T) @ V on [2048, 512] f32.

Per-core plan:
  - Load Q, K row-tiles [128, 512] f32; PE-transpose into QT/KT [d, s]
    layouts (f32, bitcast to f32r for the matmuls: 1 cycle/row at N=512).
  - Load V row-tiles, cast to bf16 (DVE) -> Vb [k, d].
  - For each of 16 q-tiles:
      mm1: scores[128, 2048] = QT_tile^T @ KT (f32r), PSUM 4 banks
      row max (DVE reduce, negated) -> exp(s - m) on ACT with fused row-sum
      P (bf16) -> DMA-xbar transpose -> PT [k, q]
      mm2: o[128, 512] = sum_k PT_tile^T @ Vb_tile (bf16)
      epilogue: out = o * (1/l) on ACT, DMA out.
"""

import os
import numpy as np

import concourse.bass as bass
import concourse.tile as tile
from concourse import bacc, mybir
from concourse.bass_utils import run_bass_kernel_spmd
from concourse.masks import make_identity

B, S, D = 8, 2048, 512
P = 128              # SBUF partitions
NQ = S // P          # 16 q tiles
ND = D // P          # 4 d tiles (contraction tiles for mm1)
KB = 512             # k block (moving free dim) for mm1
NKB = S // KB        # 4 k blocks
NKT = S // P         # 16 k tiles (contraction tiles for mm2)

F32 = mybir.dt.float32
F32R = mybir.dt.float32r
BF16 = mybir.dt.bfloat16
AX = mybir.AxisListType.X
EXP = mybir.ActivationFunctionType.Exp
COPY = mybir.ActivationFunctionType.Copy


def build_attention(tc, out_ext, q_ext, k_ext, v_ext):
    nc = tc.nc
    with (
        tc.tile_pool(name="const", bufs=1) as const_pool,
        tc.tile_pool(name="load", bufs=4) as load_pool,
        tc.tile_pool(name="persist", bufs=1) as persist_pool,
        tc.tile_pool(name="pbuf", bufs=2) as p_pool,
        tc.tile_pool(name="stats", bufs=4) as stats_pool,
        tc.tile_pool(name="osb", bufs=3) as out_pool,
        tc.tile_pool(name="psum_s", bufs=1, space="PSUM") as psum_s_pool,
        tc.tile_pool(name="psum_t", bufs=2, space="PSUM") as psum_t_pool,
        tc.tile_pool(name="psum_o", bufs=2, space="PSUM") as psum_o_pool,
    ):
        ident = const_pool.tile([P, P], F32)
        make_identity(nc, ident[:])

        # Persistent SBUF: QT/KT in [d, s] layout, Vb bf16 in [k, d] layout.
        # QT[p, j, s] = Q[s, j*128 + p]; same for KT; Vb[p, t, d] = V[t*128+p, d]
        KT = persist_pool.tile([P, ND, S], F32R)
        QT = persist_pool.tile([P, ND, S], F32R)
        Vb = persist_pool.tile([P, NKT, D], BF16)

        def load_and_transpose(src_ext, dst, tag):
            for t in range(NKT):
                tile_in = load_pool.tile([P, D], F32, tag=tag)
                nc.sync.dma_start(out=tile_in[:], in_=src_ext[t * P:(t + 1) * P, :])
                for j in range(ND):
                    ps = psum_t_pool.tile([P, P], F32, tag="tps")
                    nc.tensor.transpose(ps[:], tile_in[:, j * P:(j + 1) * P], ident[:])
                    nc.vector.tensor_copy(out=dst[:, j, t * P:(t + 1) * P], in_=ps[:])

        # K first (mm1 needs all of K), then Q, then V (needed only at mm2).
        load_and_transpose(k_ext, KT, "kload")
        load_and_transpose(q_ext, QT, "qload")
        for t in range(NKT):
            vtile = load_pool.tile([P, D], F32, tag="vload")
            nc.sync.dma_start(out=vtile[:], in_=v_ext[t * P:(t + 1) * P, :])
            nc.scalar.copy(out=Vb[:, t, :], in_=vtile[:])

        for i in range(NQ):
            # mm1: scores[q, k] for q-tile i, all 2048 k. f32r at N=512.
            ps_scores = psum_s_pool.tile([P, S], F32, tag="scores")
            for b in range(NKB):
                for j in range(ND):
                    nc.tensor.matmul(
                        ps_scores[:, b * KB:(b + 1) * KB],
                        QT[:, j, i * P:(i + 1) * P],
                        KT[:, j, b * KB:(b + 1) * KB],
                        start=(j == 0),
                        stop=(j == ND - 1),
                    )

            # Row stats + exp. negm = -max(s); P = exp(s + negm) (bf16);
            # lsum = sum of f32 exp values (fused into the ACT pass).
            negm = stats_pool.tile([P, 1], F32, tag="negm")
            nc.vector.reduce_max(out=negm[:], in_=ps_scores[:], axis=AX, negate=True)
            pexp = p_pool.tile([P, S], BF16, tag="pexp")
            lsum = stats_pool.tile([P, 1], F32, tag="lsum")
            nc.scalar.activation(
                out=pexp[:], in_=ps_scores[:], func=EXP,
                bias=negm[:], scale=1.0, accum_out=lsum[:],
            )
            linv = stats_pool.tile([P, 1], F32, tag="linv")
            nc.vector.reciprocal(linv[:], lsum[:])

            # PT[p, t*128 + q] = pexp[q, t*128 + p]  (DMA xbar transpose, bf16)
            pt = p_pool.tile([P, S], BF16, tag="pt")
            for t in range(NKT):
                nc.sync.dma_start(
                    out=pt[:, t * P:(t + 1) * P],
                    in_=pexp[:, t * P:(t + 1) * P],
                    transpose=True,
                )

            # mm2: o[q, d] = sum_k P[q, k] V[k, d] (bf16)
            ps_o = psum_o_pool.tile([P, D], F32, tag="po")
            for t in range(NKT):
                nc.tensor.matmul(
                    ps_o[:],
                    pt[:, t * P:(t + 1) * P],
                    Vb[:, t, :],
                    start=(t == 0),
                    stop=(t == NKT - 1),
                )

            # epilogue: out = o / l
            osb = out_pool.tile([P, D], F32, tag="osb")
            nc.scalar.activation(out=osb[:], in_=ps_o[:], func=COPY,
                                 bias=0.0, scale=linv[:])
            nc.sync.dma_start(out=out_ext[i * P:(i + 1) * P, :], in_=osb[:])


def build():
    nc = bacc.Bacc("TRN2", target_bir_lowering=False, debug=False,
                   num_devices=B)
    q_ext = nc.dram_tensor("query", [S, D], F32, kind="ExternalInput").ap()
    k_ext = nc.dram_tensor("key", [S, D], F32, kind="ExternalInput").ap()
    v_ext = nc.dram_tensor("value", [S, D], F32, kind="ExternalInput").ap()
    out_ext = nc.dram_tensor("out", [S, D], F32, kind="ExternalOutput").ap()

    with tile.TileContext(nc) as tc:
        build_attention(tc, out_ext, q_ext, k_ext, v_ext)
    nc.compile()
    return nc


_NC_CACHE = None


def _get_nc():
    global _NC_CACHE
    if _NC_CACHE is None:
        _NC_CACHE = build()
    return _NC_CACHE


def run(inputs: dict, trace: bool = False, tmpdir: str | None = None):
    """Run on 8 NeuronCores, one batch per core. Returns (output, results)."""
    nc = _get_nc()
    q = np.ascontiguousarray(np.asarray(inputs["query"], dtype=np.float32))
    k = np.ascontiguousarray(np.asarray(inputs["key"], dtype=np.float32))
    v = np.ascontiguousarray(np.asarray(inputs["value"], dtype=np.float32))
    in_maps = [
        {"query": q[c], "key": k[c], "value": v[c]} for c in range(B)
    ]
    res = run_bass_kernel_spmd(nc, in_maps, core_ids=list(range(B)),
                               trace=trace, tmpdir=tmpdir)
    out = np.stack([res.results[c]["out"] for c in range(B)], axis=0)
    return out, res


def kernel(**inputs) -> np.ndarray:
    trace = bool(int(os.environ.get("ATTN_TRACE", "0")))
    out, _ = run(inputs, trace=trace)
    return out


if __name__ == "__main__":
    rng = np.random.default_rng(0)
    q = rng.standard_normal((B, S, D), dtype=np.float32)
    k = rng.standard_normal((B, S, D), dtype=np.float32)
    v = rng.standard_normal((B, S, D), dtype=np.float32)
    out = kernel(query=q, key=k, value=v)
    print("out", out.shape, out.dtype)
